# revision 24
# baseline (speedup 1.0000x reference)
"""Causal self-attention (B=4, T=2048, C=1024, H=16) on 8 TRN2 NeuronCores.

Sharding: data-parallel over batch (4) x tensor-parallel over head-halves (2).
Core g handles batch g//2 and heads [8*(g%2), 8*(g%2)+8) — i.e. feature
columns [512*(g%2), 512*(g%2)+512) of the concatenated head dim.
Megatron-style: Wq/Wk/Wv column-sharded, Wp row-sharded; the host sums the
two partial y contributions per batch and adds the (bv @ Wp + bp) term
(valid because softmax rows sum to 1, so the v-bias passes through attention).

Per-core pipeline (matmuls in bf16, PSUM accumulation fp32):
  1. x^T resident in SBUF (loaded once, bf16); q^T/k^T projections per
     head-pair (feature tile of 128), v projection (hp==0 only); the
     bias-adds run on the scalar engine (Identity activation + bias AP).
  2. v stored per 128-token tile as [128 tok, 8 heads, 128] with each
     head's stationary columns laid out [ones|v]: the PV matmul then
     emits the softmax denominators (replicated x64) on partitions 0:64
     — where the fast DVE reciprocal works — and o^T on 64:128.
  3. Attention per (head-pair, 512-q-tile, 128-k-tile), causally skipped:
     S^T pair = k^T . q^T (two heads row-tiled, concurrent in the PE),
     written into one 2-bank PSUM tile; ONE exp activation covers both
     heads; the causal edge gets a single strided mask multiply over
     both heads; PV accumulates denoms+o^T in PSUM.
  4. Normalize on DVE only: rec = reciprocal_approx_fast(denom rows) at
     partition base 0, then o^T rows (PSUM, base 64) * rec (SBUF, base
     0) -> oT tiles (PSUM+SBUF inputs may have different base
     partitions; SBUF+SBUF may not).
  5. Output projection y = o . Wp_half accumulated over feature tiles,
     drained PSUM->SBUF alternately on ACT/DVE, DMA'd out as bf16 (the
     host upcasts); j-loop reversed on the last head-pair to shorten
     the kernel tail.
"""

import math

import ml_dtypes
import numpy as np

import concourse.bass as bass
import concourse.tile as tile
from concourse import bacc, mybir
from concourse.bass_utils import run_bass_kernel_spmd

B, T, C, H = 4, 2048, 1024, 16
D = C // H  # 64
N_CORES = 8
F = C // 2  # 512 features per core (8 heads)
FT = F // 128  # 4 feature tiles (head pairs) per core
CCH = C // 128  # 8 contraction chunks
NQ = T // 512  # 4 q-tiles
NKT = T // 128  # 16 k-tiles
SCALE = 1.0 / math.sqrt(D)

f32 = mybir.dt.float32
bf16 = mybir.dt.bfloat16

_cache = {}


def _build():
    nc = bacc.Bacc("TRN2", target_bir_lowering=False, debug=False,
                   num_devices=N_CORES)
    xT = nc.dram_tensor("xT", [C, T], bf16, kind="ExternalInput").ap()
    wq = nc.dram_tensor("wq", [FT, 128, CCH, 128], bf16, kind="ExternalInput").ap()
    wk = nc.dram_tensor("wk", [FT, 128, CCH, 128], bf16, kind="ExternalInput").ap()
    wv = nc.dram_tensor("wv", [C, F], bf16, kind="ExternalInput").ap()
    wp = nc.dram_tensor("wp", [F, C], bf16, kind="ExternalInput").ap()
    bqk = nc.dram_tensor("bqk", [2, F], f32, kind="ExternalInput").ap()
    cinit = nc.dram_tensor("cinit", [128, 1160], bf16, kind="ExternalInput").ap()
    y = nc.dram_tensor("y", [T, C], bf16, kind="ExternalOutput").ap()

    with tile.TileContext(nc) as tc:
        _body(tc, xT, wq, wk, wv, wp, bqk, cinit, y)
    nc.compile()
    return nc


def _body(tc, xT, wq, wk, wv, wp, bqk, cinit, y):
    nc = tc.nc
    Exp = mybir.ActivationFunctionType.Exp
    Ident = mybir.ActivationFunctionType.Identity

    pools = []

    def pool(**kw):
        p = tc.alloc_tile_pool(**kw)
        pools.append(p)
        return p

    consts = pool(name="consts", bufs=1)
    big = pool(name="big", bufs=1)
    y_pool = pool(name="ysb", bufs=2)
    wqk_pool = pool(name="wqk", bufs=1)
    qkt_pool = pool(name="qkt", bufs=2)
    v_pool = pool(name="v", bufs=1)
    pt_pool = pool(name="pt", bufs=4)
    ot_pool = pool(name="ot", bufs=1)
    norm_pool = pool(name="norm", bufs=2)
    ps_qk = pool(name="ps_qk", bufs=2, space="PSUM")
    ps_s = pool(name="ps_s", bufs=2, space="PSUM")
    ps_o = pool(name="ps_o", bufs=2, space="PSUM")

    bqk_sb = consts.tile([128, 2, FT], f32, tag="bqk")
    cinit_sb = consts.tile([128, 1160], bf16, tag="cinit")
    mask2_sb = cinit_sb[:, 904:1160]  # [tri|tri] for the 2-head mask mul

    # Resident x^T (bf16, 32KB/partition), loaded once in 512-token chunks.
    x_sb = big.tile([128, CCH, T], bf16, tag="x")
    # Resident weights (DMAs issued later, interleaved with first compute).
    wv_sb = big.tile([128, CCH, F], bf16, tag="wv")
    wp_sb = big.tile([128, FT, C], bf16, tag="wp")

    # v storage: per 128-token tile, [128 tok, 8 heads, 128]; each head's
    # stationary is [ones|v], so the PV matmul emits the softmax
    # denominators (replicated x64) on partitions 0:64 — where the fast
    # DVE reciprocal works — and o^T on partitions 64:128.
    v_tiles = []
    for tt in range(NKT):
        vt = v_pool.tile([128, H // 2, 128], bf16, tag=f"v{tt}")
        nc.vector.memset(vt[:, :, 0:D], 1.0)
        v_tiles.append(vt)

    # o^T storage split per (feature-tile, q-tile) so the output projection
    # can start as soon as a q-tile's last head-pair is normalized.
    oT_tiles = [[ot_pool.tile([128, 512], bf16, tag=f"oT{f}_{j}",
                              name=f"oT{f}_{j}") for j in range(NQ)]
                for f in range(FT)]

    xT_r = xT.rearrange("(k p) t -> p k t", p=128)

    for hp in range(FT):
        # ---- q^T / k^T projections for this head pair (128 features) ----
        wq_t = wqk_pool.tile([128, CCH, 128], bf16, tag="wq")
        nc.sync.dma_start(out=wq_t[:], in_=wq[hp])
        wk_t = wqk_pool.tile([128, CCH, 128], bf16, tag="wk")
        nc.sync.dma_start(out=wk_t[:], in_=wk[hp])
        if hp == 0:
            # x chunks for tq=0 split per-cc so the first matmul starts as
            # soon as chunk 0 lands; then the small constants, wv (needed
            # by tq=0's v projection), and the rest of x.
            for cc in range(CCH):
                nc.sync.dma_start(out=x_sb[:, cc, 0:512],
                                  in_=xT_r[:, cc, 0:512])
            nc.sync.dma_start(out=bqk_sb[:],
                              in_=bqk.rearrange("b (f p) -> p b f", p=128))
            nc.sync.dma_start(out=cinit_sb[:], in_=cinit[:])
            nc.sync.dma_start(out=wv_sb[:],
                              in_=wv.rearrange("(k p) f -> p k f", p=128))
            for tq in range(1, NQ):
                ts = slice(tq * 512, (tq + 1) * 512)
                nc.sync.dma_start(out=x_sb[:, :, ts], in_=xT_r[:, :, ts])
        if hp == 1:
            nc.sync.dma_start(out=wp_sb[:],
                              in_=wp.rearrange("(k p) c -> p k c", p=128))
        qT = qkt_pool.tile([128, T], bf16, tag="qT")
        kT = qkt_pool.tile([128, T], bf16, tag="kT")
        for tq in range(NQ):
            ts = slice(tq * 512, (tq + 1) * 512)
            psq = ps_qk.tile([128, 512], f32, tag="qk")
            for cc in range(CCH):
                nc.tensor.matmul(psq[:], wq_t[:, cc, :], x_sb[:, cc, ts],
                                 start=(cc == 0), stop=(cc == CCH - 1))
            nc.scalar.activation(qT[:, ts], psq[:], Ident,
                                 bias=bqk_sb[:, 0, hp:hp + 1])
            psk = ps_qk.tile([128, 512], f32, tag="qk")
            for cc in range(CCH):
                nc.tensor.matmul(psk[:], wk_t[:, cc, :], x_sb[:, cc, ts],
                                 start=(cc == 0), stop=(cc == CCH - 1))
            nc.scalar.activation(kT[:, ts], psk[:], Ident,
                                 bias=bqk_sb[:, 1, hp:hp + 1])
            if hp == 0:
                # ---- v projection (all 512 features) for these tokens ----
                for t4 in range(4):
                    tt = tq * 4 + t4
                    toks = slice(tq * 512 + t4 * 128, tq * 512 + (t4 + 1) * 128)
                    psv = ps_qk.tile([128, F], f32, tag="qk")
                    for cc in range(CCH):
                        nc.tensor.matmul(psv[:], x_sb[:, cc, toks],
                                         wv_sb[:, cc, :],
                                         start=(cc == 0), stop=(cc == CCH - 1))
                    nc.vector.tensor_copy(
                        v_tiles[tt][:, :, D:128],
                        psv.rearrange("p (h d) -> p h d", h=H // 2))

        # ---- attention for the two heads of this pair ----
        # reversed j on the last pair so the kernel tail ends on the small
        # j=0 block (short attention + output projection).
        j_order = range(NQ) if hp < FT - 1 else reversed(range(NQ))
        for j in j_order:
            nk = 4 * j + 4
            o_ps = [ps_o.tile([128, 512], f32, tag="o", name=f"o{h2}")
                    for h2 in range(2)]
            for i in range(nk):
                # straddle tiles (r>0) only touch q >= 128*r within the
                # q-tile; the PSUM zero-fill from the i==0 start covers the
                # untouched (causally masked) columns.
                r = i - 4 * j
                qo = 128 * r if r > 0 else 0
                sp = ps_s.tile([128, 2, 512], f32, tag="s")
                for h2 in range(2):
                    lo = h2 * 64
                    nc.tensor.matmul(sp[:, h2, qo:512],
                                     kT[lo:lo + 64, i * 128:(i + 1) * 128],
                                     qT[lo:lo + 64, j * 512 + qo:(j + 1) * 512],
                                     start=True, stop=True)
                pt = pt_pool.tile([128, 2, 512], bf16, tag="pt")
                nc.scalar.activation(pt[:, :, qo:512], sp[:, :, qo:512],
                                     Exp, scale=SCALE)
                if r >= 0:
                    # causal edge: first 128 valid columns of both heads get
                    # the triangular mask in one strided multiply (mask2_sb
                    # holds the [128,128] triangle twice, back to back).
                    nc.vector.tensor_mul(
                        pt[:, :, qo:qo + 128], pt[:, :, qo:qo + 128],
                        mask2_sb.rearrange("p (two c) -> p two c", two=2))
                h = 2 * hp
                nc.tensor.matmul(o_ps[0][:, qo:512], v_tiles[i][:, h, :],
                                 pt[:, 0, qo:512],
                                 start=(i == 0), stop=(i == nk - 1))
                nc.tensor.matmul(o_ps[1][:, qo:512], v_tiles[i][:, h + 1, :],
                                 pt[:, 1, qo:512],
                                 start=(i == 0), stop=(i == nk - 1))
            # ---- normalize: o / denom, all on DVE ----
            # both heads: denoms rows 0:64 (x64 replicated), o rows 64:128.
            # The mul mixes a PSUM operand (base 64) with SBUF operands at
            # base 0, which the ISA allows (only SBUF+SBUF inputs must
            # align); validated exact on hardware.
            rec0 = norm_pool.tile([64, 512], f32, tag="rec", name="rec0")
            nc.vector.reciprocal_approx_fast(rec0[:], o_ps[0][0:D, :])
            nc.vector.tensor_mul(oT_tiles[hp][j][0:D, :], o_ps[0][D:128, :],
                                 rec0[:])
            rec1 = norm_pool.tile([64, 512], f32, tag="rec", name="rec1")
            nc.vector.reciprocal_approx_fast(rec1[:], o_ps[1][0:D, :])
            nc.vector.tensor_mul(oT_tiles[hp][j][D:128, :], o_ps[1][D:128, :],
                                 rec1[:])

            if hp == FT - 1:
                # ---- output projection for this q-tile's tokens ----
                for t4 in range(4):
                    tt = 4 * j + t4
                    for n in range(2):
                        psy = ps_qk.tile([128, 512], f32, tag="qk")
                        for f in range(FT):
                            nc.tensor.matmul(
                                psy[:],
                                oT_tiles[f][j][:, t4 * 128:(t4 + 1) * 128],
                                wp_sb[:, f, n * 512:(n + 1) * 512],
                                start=(f == 0), stop=(f == FT - 1))
                        y_sb = y_pool.tile([128, 512], bf16, tag="ysb")
                        # drain each psy bank with BOTH ACT and DVE working
                        # on half each — the bank frees ~2x sooner, which
                        # sets the out-projection group cadence
                        nc.scalar.copy(y_sb[:, 0:256], psy[:, 0:256])
                        nc.vector.tensor_copy(y_sb[:, 256:512], psy[:, 256:512])
                        nc.sync.dma_start(
                            out=y[tt * 128:(tt + 1) * 128,
                                  n * 512:(n + 1) * 512],
                            in_=y_sb[:])

    for p in reversed(pools):
        p.release()


def make_in_maps(x, Wq, bq, Wk, bk, Wv, bv, Wp, bp):
    bf = ml_dtypes.bfloat16
    x = np.asarray(x, dtype=np.float32)
    Wq, Wk, Wv, Wp = (np.asarray(a, dtype=np.float32) for a in (Wq, Wk, Wv, Wp))
    bq, bk, bv, bp = (np.asarray(a, dtype=np.float32) for a in (bq, bk, bv, bp))
    in_maps = []
    for g in range(N_CORES):
        b, half = g // 2, g % 2
        fs = slice(half * F, (half + 1) * F)
        # [C, 128f] -> [hp, p, k, ff] with c = k*128 + p, f = hp*128 + ff
        def shuf(w):
            return np.ascontiguousarray(
                w[:, fs].reshape(CCH, 128, FT, 128).transpose(2, 1, 0, 3)
                .astype(bf))
        in_maps.append({
            "xT": np.ascontiguousarray(x[b].T.astype(bf)),
            "wq": shuf(Wq),
            "wk": shuf(Wk),
            "wv": np.ascontiguousarray(Wv[:, fs].astype(bf)),
            "wp": np.ascontiguousarray(Wp[fs, :].astype(bf)),
            "bqk": np.ascontiguousarray(np.stack([bq[fs], bk[fs]])),
            "cinit": _cinit(),
        })
    return in_maps


def _cinit():
    if "cinit" not in _cache:
        u = np.arange(896, dtype=np.float64)[None, :]
        kk = np.arange(128, dtype=np.float64)[:, None]
        m = ((u - kk - 384) >= 0).astype(np.float32)
        tri = m[:, 384:512]
        c = np.concatenate([m, np.ones((128, 8), np.float32), tri, tri],
                           axis=1)
        _cache["cinit"] = np.ascontiguousarray(c.astype(ml_dtypes.bfloat16))
    return _cache["cinit"]


def gather(results, bv, Wv, Wp, bp):
    bias_total = (np.asarray(bv, np.float32) @ np.asarray(Wp, np.float32)
                  + np.asarray(bp, np.float32))
    y = np.empty((B, T, C), dtype=np.float32)
    for b in range(B):
        y[b] = (results[2 * b]["y"].astype(np.float32)
                + results[2 * b + 1]["y"].astype(np.float32) + bias_total)
    return y


def get_nc():
    if "nc" not in _cache:
        _cache["nc"] = _build()
    return _cache["nc"]


def kernel(x, Wq, bq, Wk, bk, Wv, bv, Wp, bp):
    nc = get_nc()
    in_maps = make_in_maps(x, Wq, bq, Wk, bk, Wv, bv, Wp, bp)
    res = run_bass_kernel_spmd(nc, in_maps, list(range(N_CORES)))
    return gather(res.results, bv, Wv, Wp, bp)


# revision 28
# speedup vs baseline: 1.0287x; 1.0287x over previous
"""Causal self-attention (B=4, T=2048, C=1024, H=16) on 8 TRN2 NeuronCores.

Sharding: data-parallel over batch (4) x tensor-parallel over head-halves (2).
Core g handles batch g//2 and heads [8*(g%2), 8*(g%2)+8) — i.e. feature
columns [512*(g%2), 512*(g%2)+512) of the concatenated head dim.
Megatron-style: Wq/Wk/Wv column-sharded, Wp row-sharded; the host sums the
two partial y contributions per batch and adds the (bv @ Wp + bp) term
(valid because softmax rows sum to 1, so the v-bias passes through attention).

Per-core pipeline (matmuls in bf16, PSUM accumulation fp32):
  1. x^T resident in SBUF (loaded once, bf16); q^T/k^T projections per
     head-pair (feature tile of 128), v projection (hp==0 only); the
     bias-adds run on the scalar engine (Identity activation + bias AP).
  2. v stored per 128-token tile as [128 tok, 8 heads, 128] with each
     head's stationary columns laid out [ones|v]: the PV matmul then
     emits the softmax denominators (replicated x64) on partitions 0:64
     — where the fast DVE reciprocal works — and o^T on 64:128.
  3. Attention per (head-pair, 512-q-tile, 128-k-tile), causally skipped:
     S^T pair = k^T . q^T (two heads row-tiled, concurrent in the PE),
     written into one 2-bank PSUM tile; ONE exp activation covers both
     heads; the causal edge gets a single strided mask multiply over
     both heads; PV accumulates denoms+o^T in PSUM.
  4. Normalize on DVE only: rec = reciprocal_approx_fast(denom rows) at
     partition base 0, then o^T rows (PSUM, base 64) * rec (SBUF, base
     0) -> oT tiles (PSUM+SBUF inputs may have different base
     partitions; SBUF+SBUF may not).
  5. Output projection y = o . Wp_half accumulated over feature tiles,
     drained PSUM->SBUF alternately on ACT/DVE, DMA'd out as bf16 (the
     host upcasts); j-loop reversed on the last head-pair to shorten
     the kernel tail.
"""

import math

import ml_dtypes
import numpy as np

import concourse.bass as bass
import concourse.tile as tile
from concourse import bacc, mybir
from concourse.bass_utils import run_bass_kernel_spmd

B, T, C, H = 4, 2048, 1024, 16
D = C // H  # 64
N_CORES = 8
F = C // 2  # 512 features per core (8 heads)
FT = F // 128  # 4 feature tiles (head pairs) per core
CCH = C // 128  # 8 contraction chunks
NQ = T // 512  # 4 q-tiles
NKT = T // 128  # 16 k-tiles
SCALE = 1.0 / math.sqrt(D)

f32 = mybir.dt.float32
bf16 = mybir.dt.bfloat16

_cache = {}


def _build():
    nc = bacc.Bacc("TRN2", target_bir_lowering=False, debug=False,
                   num_devices=N_CORES)
    xT = nc.dram_tensor("xT", [C, T], bf16, kind="ExternalInput").ap()
    wq = nc.dram_tensor("wq", [FT, 128, CCH, 128], bf16, kind="ExternalInput").ap()
    wk = nc.dram_tensor("wk", [FT, 128, CCH, 128], bf16, kind="ExternalInput").ap()
    wv = nc.dram_tensor("wv", [C, F], bf16, kind="ExternalInput").ap()
    wp = nc.dram_tensor("wp", [F, C], bf16, kind="ExternalInput").ap()
    bqk = nc.dram_tensor("bqk", [2, F], f32, kind="ExternalInput").ap()
    cinit = nc.dram_tensor("cinit", [128, 1160], bf16, kind="ExternalInput").ap()
    y = nc.dram_tensor("y", [T, C], bf16, kind="ExternalOutput").ap()

    with tile.TileContext(nc) as tc:
        _body(tc, xT, wq, wk, wv, wp, bqk, cinit, y)
    nc.compile()
    return nc


def _body(tc, xT, wq, wk, wv, wp, bqk, cinit, y):
    nc = tc.nc
    Exp = mybir.ActivationFunctionType.Exp
    Ident = mybir.ActivationFunctionType.Identity

    pools = []

    def pool(**kw):
        p = tc.alloc_tile_pool(**kw)
        pools.append(p)
        return p

    consts = pool(name="consts", bufs=1)
    big = pool(name="big", bufs=1)
    y_pool = pool(name="ysb", bufs=4)
    wqk_pool = pool(name="wqk", bufs=1)
    qkt_pool = pool(name="qkt", bufs=2)
    v_pool = pool(name="v", bufs=1)
    pt_pool = pool(name="pt", bufs=4)
    ot_pool = pool(name="ot", bufs=1)
    norm_pool = pool(name="norm", bufs=2)
    ps_qk = pool(name="ps_qk", bufs=2, space="PSUM")
    ps_s = pool(name="ps_s", bufs=2, space="PSUM")
    ps_o = pool(name="ps_o", bufs=2, space="PSUM")

    bqk_sb = consts.tile([128, 2, FT], f32, tag="bqk")
    cinit_sb = consts.tile([128, 1160], bf16, tag="cinit")
    mask2_sb = cinit_sb[:, 904:1160]  # [tri|tri] for the 2-head mask mul

    # Resident x^T (bf16, 32KB/partition), loaded once in 512-token chunks.
    x_sb = big.tile([128, CCH, T], bf16, tag="x")
    # Resident weights (DMAs issued later, interleaved with first compute).
    wv_sb = big.tile([128, CCH, F], bf16, tag="wv")
    wp_sb = big.tile([128, FT, C], bf16, tag="wp")

    # v storage: per 128-token tile, [128 tok, 8 heads, 128]; each head's
    # stationary is [ones|v], so the PV matmul emits the softmax
    # denominators (replicated x64) on partitions 0:64 — where the fast
    # DVE reciprocal works — and o^T on partitions 64:128.
    v_tiles = []
    for tt in range(NKT):
        vt = v_pool.tile([128, H // 2, 128], bf16, tag=f"v{tt}")
        nc.vector.memset(vt[:, :, 0:D], 1.0)
        v_tiles.append(vt)

    # o^T storage split per (feature-tile, q-tile) so the output projection
    # can start as soon as a q-tile's last head-pair is normalized.
    oT_tiles = [[ot_pool.tile([128, 512], bf16, tag=f"oT{f}_{j}",
                              name=f"oT{f}_{j}") for j in range(NQ)]
                for f in range(FT)]

    xT_r = xT.rearrange("(k p) t -> p k t", p=128)

    for hp in range(FT):
        # ---- q^T / k^T projections for this head pair (128 features) ----
        wq_t = wqk_pool.tile([128, CCH, 128], bf16, tag="wq")
        nc.sync.dma_start(out=wq_t[:], in_=wq[hp])
        wk_t = wqk_pool.tile([128, CCH, 128], bf16, tag="wk")
        nc.sync.dma_start(out=wk_t[:], in_=wk[hp])
        if hp == 0:
            # x chunks for tq=0 split per-cc so the first matmul starts as
            # soon as chunk 0 lands; then the small constants, wv (needed
            # by tq=0's v projection), and the rest of x.
            for cc in range(CCH):
                nc.sync.dma_start(out=x_sb[:, cc, 0:512],
                                  in_=xT_r[:, cc, 0:512])
            nc.sync.dma_start(out=bqk_sb[:],
                              in_=bqk.rearrange("b (f p) -> p b f", p=128))
            nc.sync.dma_start(out=cinit_sb[:], in_=cinit[:])
            nc.sync.dma_start(out=wv_sb[:],
                              in_=wv.rearrange("(k p) f -> p k f", p=128))
            for tq in range(1, NQ):
                ts = slice(tq * 512, (tq + 1) * 512)
                nc.sync.dma_start(out=x_sb[:, :, ts], in_=xT_r[:, :, ts])
        if hp == 1:
            nc.sync.dma_start(out=wp_sb[:],
                              in_=wp.rearrange("(k p) c -> p k c", p=128))
        qT = qkt_pool.tile([128, T], bf16, tag="qT")
        kT = qkt_pool.tile([128, T], bf16, tag="kT")
        for tq in range(NQ):
            ts = slice(tq * 512, (tq + 1) * 512)
            psq = ps_qk.tile([128, 512], f32, tag="qk")
            for cc in range(CCH):
                nc.tensor.matmul(psq[:], wq_t[:, cc, :], x_sb[:, cc, ts],
                                 start=(cc == 0), stop=(cc == CCH - 1))
            nc.scalar.activation(qT[:, ts], psq[:], Ident,
                                 bias=bqk_sb[:, 0, hp:hp + 1])
            psk = ps_qk.tile([128, 512], f32, tag="qk")
            for cc in range(CCH):
                nc.tensor.matmul(psk[:], wk_t[:, cc, :], x_sb[:, cc, ts],
                                 start=(cc == 0), stop=(cc == CCH - 1))
            nc.scalar.activation(kT[:, ts], psk[:], Ident,
                                 bias=bqk_sb[:, 1, hp:hp + 1])
            if hp == 0:
                # ---- v projection (all 512 features) for these tokens ----
                for t4 in range(4):
                    tt = tq * 4 + t4
                    toks = slice(tq * 512 + t4 * 128, tq * 512 + (t4 + 1) * 128)
                    psv = ps_qk.tile([128, F], f32, tag="qk")
                    for cc in range(CCH):
                        nc.tensor.matmul(psv[:], x_sb[:, cc, toks],
                                         wv_sb[:, cc, :],
                                         start=(cc == 0), stop=(cc == CCH - 1))
                    nc.vector.tensor_copy(
                        v_tiles[tt][:, :, D:128],
                        psv.rearrange("p (h d) -> p h d", h=H // 2))

        # ---- attention for the two heads of this pair ----
        # reversed j on the last pair so the kernel tail ends on the small
        # j=0 block (short attention + output projection).
        j_order = range(NQ) if hp < FT - 1 else reversed(range(NQ))
        for j in j_order:
            nk = 4 * j + 4
            o_ps = [ps_o.tile([128, 512], f32, tag="o", name=f"o{h2}")
                    for h2 in range(2)]
            rec = [norm_pool.tile([64, 512], f32, tag="rec", name=f"rec{h2}")
                   for h2 in range(2)]

            def norm_cols(cs, ce):
                # normalize o/denom for q-columns [cs:ce).  Both heads:
                # denoms rows 0:64 (x64 replicated), o rows 64:128.  The
                # mul mixes a PSUM operand (base 64) with SBUF operands at
                # base 0, which the ISA allows (only SBUF+SBUF inputs must
                # align); validated exact on hardware.
                for h2 in range(2):
                    nc.vector.reciprocal_approx_fast(rec[h2][:, cs:ce],
                                                     o_ps[h2][0:D, cs:ce])
                    nc.vector.tensor_mul(
                        oT_tiles[hp][j][h2 * D:(h2 + 1) * D, cs:ce],
                        o_ps[h2][D:128, cs:ce], rec[h2][:, cs:ce])

            for i in range(nk):
                # straddle tiles (r>0) only touch q >= 128*r within the
                # q-tile; the PSUM zero-fill from the i==0 start covers the
                # untouched (causally masked) columns.
                r = i - 4 * j
                qo = 128 * r if r > 0 else 0
                sp = ps_s.tile([128, 2, 512], f32, tag="s")
                for h2 in range(2):
                    lo = h2 * 64
                    nc.tensor.matmul(sp[:, h2, qo:512],
                                     kT[lo:lo + 64, i * 128:(i + 1) * 128],
                                     qT[lo:lo + 64, j * 512 + qo:(j + 1) * 512],
                                     start=True, stop=True)
                pt = pt_pool.tile([128, 2, 512], bf16, tag="pt")
                nc.scalar.activation(pt[:, :, qo:512], sp[:, :, qo:512],
                                     Exp, scale=SCALE)
                if r >= 0:
                    # causal edge: first 128 valid columns of both heads get
                    # the triangular mask in one strided multiply (mask2_sb
                    # holds the [128,128] triangle twice, back to back).
                    nc.vector.tensor_mul(
                        pt[:, :, qo:qo + 128], pt[:, :, qo:qo + 128],
                        mask2_sb.rearrange("p (two c) -> p two c", two=2))
                h = 2 * hp
                nc.tensor.matmul(o_ps[0][:, qo:512], v_tiles[i][:, h, :],
                                 pt[:, 0, qo:512],
                                 start=(i == 0), stop=(i == nk - 1))
                nc.tensor.matmul(o_ps[1][:, qo:512], v_tiles[i][:, h + 1, :],
                                 pt[:, 1, qo:512],
                                 start=(i == 0), stop=(i == nk - 1))
                if hp == FT - 1 and i == 4 * j + 1:
                    # q-columns [0:256) have received their last PV
                    # contribution (later straddle tiles only touch
                    # [256:512)) — normalize them early so the output
                    # projection's critical tail shrinks.
                    norm_cols(0, 256)
            # ---- normalize (remaining columns) on DVE ----
            if hp == FT - 1:
                norm_cols(256, 512)
            else:
                norm_cols(0, 512)

            if hp == FT - 1:
                # ---- output projection for this q-tile's tokens ----
                for t4 in range(4):
                    tt = 4 * j + t4
                    for n in range(2):
                        psy = ps_qk.tile([128, 512], f32, tag="qk")
                        for f in range(FT):
                            nc.tensor.matmul(
                                psy[:],
                                oT_tiles[f][j][:, t4 * 128:(t4 + 1) * 128],
                                wp_sb[:, f, n * 512:(n + 1) * 512],
                                start=(f == 0), stop=(f == FT - 1))
                        y_sb = y_pool.tile([128, 512], bf16, tag="ysb")
                        # drain each psy bank with BOTH ACT and DVE working
                        # on half each — the bank frees ~2x sooner, which
                        # sets the out-projection group cadence
                        nc.scalar.copy(y_sb[:, 0:256], psy[:, 0:256])
                        nc.vector.tensor_copy(y_sb[:, 256:512], psy[:, 256:512])
                        # issue the y DMA from the otherwise-idle gpsimd
                        # queue so triggers don't serialize behind the x/w
                        # input DMAs on the sync queue
                        nc.gpsimd.dma_start(
                            out=y[tt * 128:(tt + 1) * 128,
                                  n * 512:(n + 1) * 512],
                            in_=y_sb[:])

    for p in reversed(pools):
        p.release()


def make_in_maps(x, Wq, bq, Wk, bk, Wv, bv, Wp, bp):
    bf = ml_dtypes.bfloat16
    x = np.asarray(x, dtype=np.float32)
    Wq, Wk, Wv, Wp = (np.asarray(a, dtype=np.float32) for a in (Wq, Wk, Wv, Wp))
    bq, bk, bv, bp = (np.asarray(a, dtype=np.float32) for a in (bq, bk, bv, bp))
    in_maps = []
    for g in range(N_CORES):
        b, half = g // 2, g % 2
        fs = slice(half * F, (half + 1) * F)
        # [C, 128f] -> [hp, p, k, ff] with c = k*128 + p, f = hp*128 + ff
        def shuf(w):
            return np.ascontiguousarray(
                w[:, fs].reshape(CCH, 128, FT, 128).transpose(2, 1, 0, 3)
                .astype(bf))
        in_maps.append({
            "xT": np.ascontiguousarray(x[b].T.astype(bf)),
            "wq": shuf(Wq),
            "wk": shuf(Wk),
            "wv": np.ascontiguousarray(Wv[:, fs].astype(bf)),
            "wp": np.ascontiguousarray(Wp[fs, :].astype(bf)),
            "bqk": np.ascontiguousarray(np.stack([bq[fs], bk[fs]])),
            "cinit": _cinit(),
        })
    return in_maps


def _cinit():
    if "cinit" not in _cache:
        u = np.arange(896, dtype=np.float64)[None, :]
        kk = np.arange(128, dtype=np.float64)[:, None]
        m = ((u - kk - 384) >= 0).astype(np.float32)
        tri = m[:, 384:512]
        c = np.concatenate([m, np.ones((128, 8), np.float32), tri, tri],
                           axis=1)
        _cache["cinit"] = np.ascontiguousarray(c.astype(ml_dtypes.bfloat16))
    return _cache["cinit"]


def gather(results, bv, Wv, Wp, bp):
    bias_total = (np.asarray(bv, np.float32) @ np.asarray(Wp, np.float32)
                  + np.asarray(bp, np.float32))
    y = np.empty((B, T, C), dtype=np.float32)
    for b in range(B):
        y[b] = (results[2 * b]["y"].astype(np.float32)
                + results[2 * b + 1]["y"].astype(np.float32) + bias_total)
    return y


def get_nc():
    if "nc" not in _cache:
        _cache["nc"] = _build()
    return _cache["nc"]


def kernel(x, Wq, bq, Wk, bk, Wv, bv, Wp, bp):
    nc = get_nc()
    in_maps = make_in_maps(x, Wq, bq, Wk, bk, Wv, bv, Wp, bp)
    res = run_bass_kernel_spmd(nc, in_maps, list(range(N_CORES)))
    return gather(res.results, bv, Wv, Wp, bp)
